# revision 7
# baseline (speedup 1.0000x reference)
"""Trainium2 Bass kernel for nn_ExRestSelfAtten (windowed local attention).

Self-contained: hardcodes shapes, shards batch across 8 NeuronCores (pure data
parallel), runs a Tile/Bass kernel per core, reassembles full outputs on host.

Math (validated vs reference in fp64/fp32 numpy, err ~3e-6):
  h  = relu(x@W1 + b1) + pe                      (feature-major hT on device)
  scores(s,s') = (h[s]@A) . h[s'],  A = Wq Wk^T / sqrt(H)
  banded softmax over s' in [s-2, s+2] with zero-padding semantics
  context @ Wv folded:  ctx_h = sum_m w_m h[s+2-m];  W2' = Wv@W2
  out = relu(ctx@W2' + b2')@W3 + b3
Key trick: with masked exp in j-major layout expT[j,s] (zeros off-band), the
5-band extraction is a matmul with Ind[j,r] = [j==r mod 5] (5 consecutive
in-band j's have distinct residues mod 5); a ones column gives the softmax
denominator. The per-row cyclic permutation of the 5 residues and the final
division by the denominator are undone on the host at gather time.
"""

import numpy as np

import concourse.bacc as bacc
import concourse.bass as bass
import concourse.mybir as mybir
from concourse import tile
from concourse.bass_utils import run_bass_kernel_spmd

F32 = mybir.dt.float32
F32R = mybir.dt.float32r

B, S, IN, H, MID, OUT = 1024, 100, 100, 128, 32, 2
ATTEN = 2
J = S + 2 * ATTEN          # 104 padded neighbor positions
NCORES = 8
BS = B // NCORES           # 128 sequences per core
GRP = 4                    # sequences per inner group (one PSUM bank of scores)
NGRP = BS // GRP           # 32 groups per core
TT = GRP * S               # 400 tokens per group
SQRT_H = float(np.sqrt(float(H)))

AF = mybir.ActivationFunctionType
ALU = mybir.AluOpType


def build_nc(use_b2p: bool, use_f32r: bool = True):
    DT = F32R if use_f32r else F32
    nc = bacc.Bacc("TRN2", num_devices=NCORES)

    xT = nc.dram_tensor("xT", [IN + 1, BS * S], DT, kind="ExternalInput")
    w1aug_d = nc.dram_tensor("w1aug", [IN + 1, H], DT, kind="ExternalInput")
    a_d = nc.dram_tensor("amat", [H, H], DT, kind="ExternalInput")
    w2p_d = nc.dram_tensor("w2p", [H, MID], DT, kind="ExternalInput")
    w3blk_d = nc.dram_tensor("w3blk", [H, GRP * OUT], DT, kind="ExternalInput")
    pet_d = nc.dram_tensor("pet", [H, S], F32, kind="ExternalInput")
    ind6_d = nc.dram_tensor("ind6", [J, 6], DT, kind="ExternalInput")
    b2rep_d = nc.dram_tensor("b2rep", [J, MID], DT, kind="ExternalInput")

    w5_d = nc.dram_tensor("w5", [NGRP, 6, TT], F32, kind="ExternalOutput")
    m2_d = nc.dram_tensor("m2", [NGRP, GRP * OUT, S], F32, kind="ExternalOutput")

    with tile.TileContext(nc) as tc:
        with (
            tc.tile_pool(name="consts", bufs=1) as cpool,
            tc.tile_pool(name="data", bufs=2) as dpool,
            tc.tile_pool(name="psum", bufs=1, space="PSUM") as ppool,
        ):
            w1aug = cpool.tile([IN + 1, H], DT)
            amat = cpool.tile([H, H], DT)
            w2p = cpool.tile([H, MID], DT)
            w3blk = cpool.tile([H, GRP * OUT], DT)
            pet = cpool.tile([H, S], F32)
            ind6 = cpool.tile([J, 6], DT)
            nc.sync.dma_start(w1aug, w1aug_d[:])
            nc.sync.dma_start(amat, a_d[:])
            nc.sync.dma_start(w2p, w2p_d[:])
            nc.sync.dma_start(w3blk, w3blk_d[:])
            nc.sync.dma_start(pet, pet_d[:])
            nc.sync.dma_start(ind6, ind6_d[:])
            if use_b2p:
                b2rep = cpool.tile([J, MID], DT)
                nc.sync.dma_start(b2rep, b2rep_d[:])

            for g in range(NGRP):
                t0 = g * TT
                xt = dpool.tile([IN + 1, TT], DT, tag="xt")
                nc.sync.dma_start(xt, xT[:, t0 : t0 + TT])

                # mm1: hT = relu(W1aug^T @ xT) (+pe below), feature-major
                ph = ppool.tile([H, GRP, S], F32, tag="ph")
                nc.tensor.matmul(
                    ph,
                    w1aug,
                    xt,
                    start=True,
                    stop=True,
                )
                # padded hT: [128, GRP, 104] with zero pad cols 0,1,102,103
                htp = dpool.tile([H, GRP, J], DT, tag="htp")
                nc.vector.memset(htp[:, :, 0:2], 0.0)
                nc.vector.memset(htp[:, :, J - 2 : J], 0.0)
                nc.scalar.activation(htp[:, :, 2 : 2 + S], ph, AF.Relu)
                nc.vector.tensor_add(
                    htp[:, :, 2 : 2 + S],
                    htp[:, :, 2 : 2 + S],
                    pet.unsqueeze(1).broadcast_to([H, GRP, S]),
                )

                # mm2: gT = A^T @ hT
                pg = ppool.tile([H, GRP, S], F32, tag="pg")
                nc.tensor.matmul(
                    pg,
                    amat,
                    htp[:, :, 2 : 2 + S],
                    start=True,
                    stop=True,
                )
                gt = dpool.tile([H, GRP, S], DT, tag="gt")
                nc.vector.tensor_copy(gt, pg)

                # mm3 (per seq): scoresT[j, s] = hpad[j] . g[s]
                psc = ppool.tile([J, GRP, S], F32, tag="psc")
                for i in range(GRP):
                    nc.tensor.matmul(
                        psc[:, i, :],
                        htp[:, i, :],
                        gt[:, i, :],
                        start=True,
                        stop=True,
                    )
                # exp (scores bounded ~11, no max-sub needed), then band mask:
                # keep iff s <= j <= s+4  (two affine selects on gpsimd)
                expa = dpool.tile([J, GRP, S], DT, tag="expa")
                expb = dpool.tile([J, GRP, S], DT, tag="expb")
                nc.scalar.activation(expa, psc, AF.Exp)
                nc.gpsimd.affine_select(
                    expb,
                    expa,
                    pattern=[[0, GRP], [1, S]],
                    compare_op=ALU.is_ge,
                    fill=0.0,
                    base=4,
                    channel_multiplier=-1,
                )
                nc.gpsimd.affine_select(
                    expa,
                    expb,
                    pattern=[[0, GRP], [-1, S]],
                    compare_op=ALU.is_ge,
                    fill=0.0,
                    base=0,
                    channel_multiplier=1,
                )

                # mm4: W5 = Ind6^T @ expT -> [6, TT] (5 permuted band rows + denom)
                pw5 = ppool.tile([6, GRP, S], F32, tag="pw5")
                nc.tensor.matmul(
                    pw5,
                    ind6,
                    expa.rearrange("p a b -> p (a b)"),
                    start=True,
                    stop=True,
                )
                w5s = dpool.tile([6, GRP, S], F32, tag="w5s")
                nc.vector.tensor_copy(w5s, pw5)
                nc.sync.dma_start(w5_d[g], w5s.rearrange("p a b -> p (a b)"))

                # mm5 (per seq): hw2[j, mid] = hpad[j] @ W2p
                phw2 = ppool.tile([J, GRP * MID], F32, tag="phw2")
                for i in range(GRP):
                    nc.tensor.matmul(
                        phw2[:, i * MID : (i + 1) * MID],
                        htp[:, i, :],
                        w2p,
                        start=True,
                        stop=True,
                    )
                hw2 = dpool.tile([J, GRP * MID], DT, tag="hw2")
                nc.scalar.copy(hw2, phw2)

                # mm6 (per seq, col-group packed): mlp1T[mid, s] = hw2^T @ expT
                pm1 = ppool.tile([GRP * MID, S], F32, tag="pm1")
                for i in range(GRP):
                    nc.tensor.matmul(
                        pm1[i * MID : (i + 1) * MID, :],
                        hw2[:, i * MID : (i + 1) * MID],
                        expa[:, i, :],
                        start=True,
                        stop=not use_b2p,
                        tile_position=(0, i * MID),
                    )
                    if use_b2p:
                        # += b2p[mid] * denom[s] so host's /denom yields +b2p
                        nc.tensor.matmul(
                            pm1[i * MID : (i + 1) * MID, :],
                            b2rep,
                            expa[:, i, :],
                            start=False,
                            stop=True,
                            tile_position=(0, i * MID),
                        )
                r1 = dpool.tile([GRP * MID, S], DT, tag="r1")
                nc.scalar.activation(r1, pm1, AF.Relu)

                # mm7: m2 = W3blk^T @ r1 -> [GRP*OUT, S]
                pm2 = ppool.tile([GRP * OUT, S], F32, tag="pm2")
                nc.tensor.matmul(pm2, w3blk, r1, start=True, stop=True)
                m2s = dpool.tile([GRP * OUT, S], F32, tag="m2s")
                nc.scalar.copy(m2s, pm2)
                nc.sync.dma_start(m2_d[g], m2s)

    nc.compile()
    return nc


_CACHE = {}


USE_F32R = True


def _get_nc(use_b2p: bool):
    key = (use_b2p, USE_F32R)
    if key not in _CACHE:
        _CACHE[key] = build_nc(use_b2p, USE_F32R)
    return _CACHE[key]


def _host_prep(inputs):
    x = np.asarray(inputs["x"], np.float32)
    W1 = np.asarray(inputs["W1"], np.float32)
    b1 = np.asarray(inputs["b1"], np.float32).reshape(H)
    Wq = np.asarray(inputs["Wq"], np.float32)
    Wk = np.asarray(inputs["Wk"], np.float32)
    Wv = np.asarray(inputs["Wv"], np.float32)
    bv = np.asarray(inputs["bv"], np.float32).reshape(H)
    W2 = np.asarray(inputs["W2"], np.float32)
    b2 = np.asarray(inputs["b2"], np.float32).reshape(MID)
    W3 = np.asarray(inputs["W3"], np.float32)
    b3 = np.asarray(inputs["b3"], np.float32).reshape(OUT)
    pe = np.asarray(inputs["pe"], np.float32)[0]

    A = (Wq @ Wk.T) / SQRT_H
    W2p = Wv @ W2
    b2p = bv @ W2 + b2
    W1aug = np.ascontiguousarray(np.concatenate([W1, b1[None, :]], 0))
    W3blk = np.zeros((H, GRP * OUT), np.float32)
    for i in range(GRP):
        W3blk[MID * i : MID * i + MID, OUT * i : OUT * i + OUT] = W3
    ind6 = np.zeros((J, 6), np.float32)
    ind6[np.arange(J), np.arange(J) % 5] = 1.0
    ind6[:, 5] = 1.0
    peT = np.ascontiguousarray(pe[:S].T)
    b2rep = np.ascontiguousarray(np.broadcast_to(b2p[None, :], (J, MID)))

    consts = {
        "w1aug": W1aug,
        "amat": np.ascontiguousarray(A),
        "w2p": np.ascontiguousarray(W2p),
        "w3blk": W3blk,
        "pet": peT,
        "ind6": ind6,
        "b2rep": b2rep,
    }
    use_b2p = bool(np.any(b2p != 0.0))

    in_maps = []
    for c in range(NCORES):
        xs = x[c * BS : (c + 1) * BS].reshape(BS * S, IN).T
        xT_aug = np.ascontiguousarray(
            np.concatenate([xs, np.ones((1, BS * S), np.float32)], 0)
        )
        m = {"xT": xT_aug}
        m.update(consts)
        in_maps.append(m)
    return in_maps, use_b2p, b3


# index table undoing the mod-5 residue permutation: w[s, m] = W5[(s+4-m)%5, s]
_RIDX = ((np.arange(S)[:, None] + 4 - np.arange(5)[None, :]) % 5).astype(np.int64)


def _host_post(results, b3):
    # per core: w5 [NGRP, 6, TT], m2 [NGRP, GRP*OUT, S]
    w5 = np.stack([np.asarray(r["w5"]) for r in results])  # [NC, NGRP, 6, TT]
    m2 = np.stack([np.asarray(r["m2"]) for r in results])  # [NC, NGRP, 8, S]

    w5 = w5.reshape(NCORES, NGRP, 6, GRP, S).transpose(0, 1, 3, 2, 4)
    w5 = w5.reshape(B, 6, S)                     # [b, r, s]
    denom = w5[:, 5, :]                          # [b, s]
    # band: w[b, s, m] = w5[b, (s+4-m)%5, s] / denom
    sidx = np.arange(S)[:, None]
    wband = w5[:, _RIDX, sidx] / denom[:, :, None]   # [B, S, 5]

    m2 = m2.reshape(NCORES, NGRP, GRP, OUT, S).transpose(0, 1, 2, 4, 3)
    m2 = m2.reshape(B, S, OUT)
    out = m2 / denom[:, :, None] + b3[None, None, :]

    return out.astype(np.float32), wband.astype(np.float32)


def kernel(**inputs):
    in_maps, use_b2p, b3 = _host_prep(inputs)
    nc = _get_nc(use_b2p)
    res = run_bass_kernel_spmd(nc, in_maps, list(range(NCORES)))
    out, wband = _host_post(res.results, b3)
    return out, wband[:, :, None, :]


# revision 9
# speedup vs baseline: 1.0255x; 1.0255x over previous
"""Trainium2 Bass kernel for nn_ExRestSelfAtten (windowed local attention).

Self-contained: hardcodes shapes, shards batch across 8 NeuronCores (pure data
parallel), runs a Tile/Bass kernel per core, reassembles full outputs on host.

Math (validated vs reference in fp64/fp32 numpy, err ~3e-6):
  h  = relu(x@W1 + b1) + pe                      (feature-major hT on device)
  scores(s,s') = (h[s]@A) . h[s'],  A = Wq Wk^T / sqrt(H)
  banded softmax over s' in [s-2, s+2] with zero-padding semantics
  context @ Wv folded:  ctx_h = sum_m w_m h[s+2-m];  W2' = Wv@W2
  out = relu(ctx@W2' + b2')@W3 + b3
Key trick: with masked exp in j-major layout expT[j,s] (zeros off-band), the
5-band extraction is a matmul with Ind[j,r] = [j==r mod 5] (5 consecutive
in-band j's have distinct residues mod 5); a ones column gives the softmax
denominator. The per-row cyclic permutation of the 5 residues and the final
division by the denominator are undone on the host at gather time.
"""

import numpy as np

import concourse.bacc as bacc
import concourse.bass as bass
import concourse.mybir as mybir
from concourse import tile
from concourse.bass_utils import run_bass_kernel_spmd

F32 = mybir.dt.float32
F32R = mybir.dt.float32r

B, S, IN, H, MID, OUT = 1024, 100, 100, 128, 32, 2
ATTEN = 2
J = S + 2 * ATTEN          # 104 padded neighbor positions
NCORES = 8
BS = B // NCORES           # 128 sequences per core
GRP = 4                    # sequences per inner group (one PSUM bank of scores)
NGRP = BS // GRP           # 32 groups per core
TT = GRP * S               # 400 tokens per group
SQRT_H = float(np.sqrt(float(H)))

AF = mybir.ActivationFunctionType
ALU = mybir.AluOpType


def build_nc(use_b2p: bool, use_f32r: bool = True):
    DT = F32R if use_f32r else F32
    nc = bacc.Bacc("TRN2", num_devices=NCORES)

    xT = nc.dram_tensor("xT", [IN + 1, BS * S], DT, kind="ExternalInput")
    w1aug_d = nc.dram_tensor("w1aug", [IN + 1, H], DT, kind="ExternalInput")
    a_d = nc.dram_tensor("amat", [H, H], DT, kind="ExternalInput")
    w2p_d = nc.dram_tensor("w2p", [H, MID], F32, kind="ExternalInput")
    w3blk_d = nc.dram_tensor("w3blk", [H, GRP * OUT], F32, kind="ExternalInput")
    pet_d = nc.dram_tensor("pet", [H, S], F32, kind="ExternalInput")
    ind6_d = nc.dram_tensor("ind6", [J, 6], DT, kind="ExternalInput")
    zpad_d = nc.dram_tensor("zpad", [H, 2], DT, kind="ExternalInput")
    b2rep_d = nc.dram_tensor("b2rep", [J, MID], F32, kind="ExternalInput")

    w5_d = nc.dram_tensor("w5", [NGRP, 6, TT], F32, kind="ExternalOutput")
    m2_d = nc.dram_tensor("m2", [NGRP, GRP * OUT, S], F32, kind="ExternalOutput")

    with tile.TileContext(nc) as tc:
        with (
            tc.tile_pool(name="consts", bufs=1) as cpool,
            tc.tile_pool(name="data", bufs=2) as dpool,
            tc.tile_pool(name="psum", bufs=1, space="PSUM") as ppool,
        ):
            w1aug = cpool.tile([IN + 1, H], DT)
            amat = cpool.tile([H, H], DT)
            w2p = cpool.tile([H, MID], F32)
            w3blk = cpool.tile([H, GRP * OUT], F32)
            pet = cpool.tile([H, S], F32)
            ind6 = cpool.tile([J, 6], DT)
            nc.sync.dma_start(w1aug, w1aug_d[:])
            nc.sync.dma_start(amat, a_d[:])
            nc.sync.dma_start(w2p, w2p_d[:])
            nc.sync.dma_start(w3blk, w3blk_d[:])
            nc.sync.dma_start(pet, pet_d[:])
            nc.sync.dma_start(ind6, ind6_d[:])
            zpad = cpool.tile([H, 2], DT)
            nc.sync.dma_start(zpad, zpad_d[:])
            if use_b2p:
                b2rep = cpool.tile([J, MID], F32)
                nc.sync.dma_start(b2rep, b2rep_d[:])

            for g in range(NGRP):
                t0 = g * TT
                xt = dpool.tile([IN + 1, TT], DT, tag="xt")
                nc.sync.dma_start(xt, xT[:, t0 : t0 + TT])

                # mm1: hT = relu(W1aug^T @ xT) (+pe below), feature-major
                ph = ppool.tile([H, GRP, S], F32, tag="ph")
                nc.tensor.matmul(
                    ph,
                    w1aug,
                    xt,
                    start=True,
                    stop=True,
                )
                # padded hT: [128, GRP, 104] with zero pad cols 0,1,102,103
                htp = dpool.tile([H, GRP, J], DT, tag="htp")
                zb = zpad.unsqueeze(1).broadcast_to([H, GRP, 2])
                nc.vector.tensor_copy(htp[:, :, 0:2], zb)
                nc.vector.tensor_copy(htp[:, :, J - 2 : J], zb)
                nc.scalar.activation(htp[:, :, 2 : 2 + S], ph, AF.Relu)
                nc.vector.tensor_add(
                    htp[:, :, 2 : 2 + S],
                    htp[:, :, 2 : 2 + S],
                    pet.unsqueeze(1).broadcast_to([H, GRP, S]),
                )

                # mm2: gT = A^T @ hT
                pg = ppool.tile([H, GRP, S], F32, tag="pg")
                nc.tensor.matmul(
                    pg,
                    amat,
                    htp[:, :, 2 : 2 + S],
                    start=True,
                    stop=True,
                )
                gt = dpool.tile([H, GRP, S], DT, tag="gt")
                nc.vector.tensor_copy(gt, pg)

                # mm3 (per seq): scoresT[j, s] = hpad[j] . g[s]
                psc = ppool.tile([J, GRP, S], F32, tag="psc")
                for i in range(GRP):
                    nc.tensor.matmul(
                        psc[:, i, :],
                        htp[:, i, :],
                        gt[:, i, :],
                        start=True,
                        stop=True,
                    )
                # exp (scores bounded ~11, no max-sub needed), then band mask:
                # keep iff s <= j <= s+4  (two affine selects on gpsimd)
                expa = dpool.tile([J, GRP, S], DT, tag="expa")
                expb = dpool.tile([J, GRP, S], DT, tag="expb")
                nc.scalar.activation(expa, psc, AF.Exp)
                nc.gpsimd.affine_select(
                    expb,
                    expa,
                    pattern=[[0, GRP], [1, S]],
                    compare_op=ALU.is_ge,
                    fill=0.0,
                    base=4,
                    channel_multiplier=-1,
                )
                nc.gpsimd.affine_select(
                    expa,
                    expb,
                    pattern=[[0, GRP], [-1, S]],
                    compare_op=ALU.is_ge,
                    fill=0.0,
                    base=0,
                    channel_multiplier=1,
                )

                # mm4: W5 = Ind6^T @ expT -> [6, TT] (5 permuted band rows + denom)
                pw5 = ppool.tile([6, GRP, S], F32, tag="pw5")
                nc.tensor.matmul(
                    pw5,
                    ind6,
                    expa.rearrange("p a b -> p (a b)"),
                    start=True,
                    stop=True,
                )
                w5s = dpool.tile([6, GRP, S], F32, tag="w5s")
                nc.vector.tensor_copy(w5s, pw5)
                nc.sync.dma_start(w5_d[g], w5s.rearrange("p a b -> p (a b)"))

                # mm5 (per seq): hw2[j, mid] = hpad[j] @ W2p
                phw2 = ppool.tile([J, GRP * MID], F32, tag="phw2")
                for i in range(GRP):
                    nc.tensor.matmul(
                        phw2[:, i * MID : (i + 1) * MID],
                        htp[:, i, :].bitcast(F32),
                        w2p,
                        start=True,
                        stop=True,
                    )
                hw2 = dpool.tile([J, GRP * MID], F32, tag="hw2")
                nc.scalar.copy(hw2, phw2)

                # mm6 (per seq, col-group packed): mlp1T[mid, s] = hw2^T @ expT
                pm1 = ppool.tile([GRP * MID, S], F32, tag="pm1")
                for i in range(GRP):
                    nc.tensor.matmul(
                        pm1[i * MID : (i + 1) * MID, :],
                        hw2[:, i * MID : (i + 1) * MID],
                        expa[:, i, :].bitcast(F32),
                        start=True,
                        stop=not use_b2p,
                        tile_position=(0, i * MID),
                    )
                    if use_b2p:
                        # += b2p[mid] * denom[s] so host's /denom yields +b2p
                        nc.tensor.matmul(
                            pm1[i * MID : (i + 1) * MID, :],
                            b2rep,
                            expa[:, i, :].bitcast(F32),
                            start=False,
                            stop=True,
                            tile_position=(0, i * MID),
                        )
                r1 = dpool.tile([GRP * MID, S], F32, tag="r1")
                nc.scalar.activation(r1, pm1, AF.Relu)

                # mm7: m2 = W3blk^T @ r1 -> [GRP*OUT, S]
                pm2 = ppool.tile([GRP * OUT, S], F32, tag="pm2")
                nc.tensor.matmul(pm2, w3blk, r1, start=True, stop=True)
                m2s = dpool.tile([GRP * OUT, S], F32, tag="m2s")
                nc.scalar.copy(m2s, pm2)
                nc.sync.dma_start(m2_d[g], m2s)

    nc.compile()
    return nc


_CACHE = {}


USE_F32R = True


def _get_nc(use_b2p: bool):
    key = (use_b2p, USE_F32R)
    if key not in _CACHE:
        _CACHE[key] = build_nc(use_b2p, USE_F32R)
    return _CACHE[key]


def _host_prep(inputs):
    x = np.asarray(inputs["x"], np.float32)
    W1 = np.asarray(inputs["W1"], np.float32)
    b1 = np.asarray(inputs["b1"], np.float32).reshape(H)
    Wq = np.asarray(inputs["Wq"], np.float32)
    Wk = np.asarray(inputs["Wk"], np.float32)
    Wv = np.asarray(inputs["Wv"], np.float32)
    bv = np.asarray(inputs["bv"], np.float32).reshape(H)
    W2 = np.asarray(inputs["W2"], np.float32)
    b2 = np.asarray(inputs["b2"], np.float32).reshape(MID)
    W3 = np.asarray(inputs["W3"], np.float32)
    b3 = np.asarray(inputs["b3"], np.float32).reshape(OUT)
    pe = np.asarray(inputs["pe"], np.float32)[0]

    A = (Wq @ Wk.T) / SQRT_H
    W2p = Wv @ W2
    b2p = bv @ W2 + b2
    W1aug = np.ascontiguousarray(np.concatenate([W1, b1[None, :]], 0))
    W3blk = np.zeros((H, GRP * OUT), np.float32)
    for i in range(GRP):
        W3blk[MID * i : MID * i + MID, OUT * i : OUT * i + OUT] = W3
    ind6 = np.zeros((J, 6), np.float32)
    ind6[np.arange(J), np.arange(J) % 5] = 1.0
    ind6[:, 5] = 1.0
    peT = np.ascontiguousarray(pe[:S].T)
    b2rep = np.ascontiguousarray(np.broadcast_to(b2p[None, :], (J, MID)))

    consts = {
        "zpad": np.zeros((H, 2), np.float32),
        "w1aug": W1aug,
        "amat": np.ascontiguousarray(A),
        "w2p": np.ascontiguousarray(W2p),
        "w3blk": W3blk,
        "pet": peT,
        "ind6": ind6,
        "b2rep": b2rep,
    }
    use_b2p = bool(np.any(b2p != 0.0))

    in_maps = []
    for c in range(NCORES):
        xs = x[c * BS : (c + 1) * BS].reshape(BS * S, IN).T
        xT_aug = np.ascontiguousarray(
            np.concatenate([xs, np.ones((1, BS * S), np.float32)], 0)
        )
        m = {"xT": xT_aug}
        m.update(consts)
        in_maps.append(m)
    return in_maps, use_b2p, b3


# index table undoing the mod-5 residue permutation: w[s, m] = W5[(s+4-m)%5, s]
_RIDX = ((np.arange(S)[:, None] + 4 - np.arange(5)[None, :]) % 5).astype(np.int64)


def _host_post(results, b3):
    # per core: w5 [NGRP, 6, TT], m2 [NGRP, GRP*OUT, S]
    w5 = np.stack([np.asarray(r["w5"]) for r in results])  # [NC, NGRP, 6, TT]
    m2 = np.stack([np.asarray(r["m2"]) for r in results])  # [NC, NGRP, 8, S]

    w5 = w5.reshape(NCORES, NGRP, 6, GRP, S).transpose(0, 1, 3, 2, 4)
    w5 = w5.reshape(B, 6, S)                     # [b, r, s]
    denom = w5[:, 5, :]                          # [b, s]
    # band: w[b, s, m] = w5[b, (s+4-m)%5, s] / denom
    sidx = np.arange(S)[:, None]
    wband = w5[:, _RIDX, sidx] / denom[:, :, None]   # [B, S, 5]

    m2 = m2.reshape(NCORES, NGRP, GRP, OUT, S).transpose(0, 1, 2, 4, 3)
    m2 = m2.reshape(B, S, OUT)
    out = m2 / denom[:, :, None] + b3[None, None, :]

    return out.astype(np.float32), wband.astype(np.float32)


def kernel(**inputs):
    in_maps, use_b2p, b3 = _host_prep(inputs)
    nc = _get_nc(use_b2p)
    res = run_bass_kernel_spmd(nc, in_maps, list(range(NCORES)))
    out, wband = _host_post(res.results, b3)
    return out, wband[:, :, None, :]


# revision 10
# speedup vs baseline: 1.0427x; 1.0168x over previous
"""Trainium2 Bass kernel for nn_ExRestSelfAtten (windowed local attention).

Self-contained: hardcodes shapes, shards batch across 8 NeuronCores (pure data
parallel), runs a Tile/Bass kernel per core, reassembles full outputs on host.

Math (validated vs reference in fp64/fp32 numpy, err ~3e-6):
  h  = relu(x@W1 + b1) + pe                      (feature-major hT on device)
  scores(s,s') = (h[s]@A) . h[s'],  A = Wq Wk^T / sqrt(H)
  banded softmax over s' in [s-2, s+2] with zero-padding semantics
  context @ Wv folded:  ctx_h = sum_m w_m h[s+2-m];  W2' = Wv@W2
  out = relu(ctx@W2' + b2')@W3 + b3
Key trick: with masked exp in j-major layout expT[j,s] (zeros off-band), the
5-band extraction is a matmul with Ind[j,r] = [j==r mod 5] (5 consecutive
in-band j's have distinct residues mod 5); a ones column gives the softmax
denominator. The per-row cyclic permutation of the 5 residues and the final
division by the denominator are undone on the host at gather time.
"""

import numpy as np

import concourse.bacc as bacc
import concourse.bass as bass
import concourse.mybir as mybir
from concourse import tile
from concourse.bass_utils import run_bass_kernel_spmd

F32 = mybir.dt.float32
F32R = mybir.dt.float32r

B, S, IN, H, MID, OUT = 1024, 100, 100, 128, 32, 2
ATTEN = 2
J = S + 2 * ATTEN          # 104 padded neighbor positions
NCORES = 8
BS = B // NCORES           # 128 sequences per core
GRP = 4                    # sequences per inner group (one PSUM bank of scores)
NGRP = BS // GRP           # 32 groups per core
TT = GRP * S               # 400 tokens per group
SQRT_H = float(np.sqrt(float(H)))

AF = mybir.ActivationFunctionType
ALU = mybir.AluOpType


def build_nc(use_b2p: bool, use_f32r: bool = True):
    DT = F32R if use_f32r else F32
    nc = bacc.Bacc("TRN2", num_devices=NCORES)

    xT = nc.dram_tensor("xT", [IN + 1, BS * S], DT, kind="ExternalInput")
    w1aug_d = nc.dram_tensor("w1aug", [IN + 1, H], DT, kind="ExternalInput")
    a_d = nc.dram_tensor("amat", [H, H], DT, kind="ExternalInput")
    w2p_d = nc.dram_tensor("w2p", [H, MID], F32, kind="ExternalInput")
    w3blk_d = nc.dram_tensor("w3blk", [H, GRP * OUT], F32, kind="ExternalInput")
    pet_d = nc.dram_tensor("pet", [H, S], F32, kind="ExternalInput")
    ind6_d = nc.dram_tensor("ind6", [J, 6], DT, kind="ExternalInput")
    zpad_d = nc.dram_tensor("zpad", [H, 2], DT, kind="ExternalInput")
    b2rep_d = nc.dram_tensor("b2rep", [J, MID], F32, kind="ExternalInput")

    w5_d = nc.dram_tensor("w5", [NGRP, 6, TT], F32, kind="ExternalOutput")
    m2_d = nc.dram_tensor("m2", [NGRP, GRP * OUT, S], F32, kind="ExternalOutput")

    with tile.TileContext(nc) as tc:
        with (
            tc.tile_pool(name="consts", bufs=1) as cpool,
            tc.tile_pool(name="data", bufs=2) as dpool,
            tc.tile_pool(name="psum", bufs=1, space="PSUM") as ppool,
        ):
            w1aug = cpool.tile([IN + 1, H], DT)
            amat = cpool.tile([H, H], DT)
            w2p = cpool.tile([H, MID], F32)
            w3blk = cpool.tile([H, GRP * OUT], F32)
            pet = cpool.tile([H, S], F32)
            ind6 = cpool.tile([J, 6], DT)
            nc.sync.dma_start(w1aug, w1aug_d[:])
            nc.sync.dma_start(amat, a_d[:])
            nc.sync.dma_start(w2p, w2p_d[:])
            nc.sync.dma_start(w3blk, w3blk_d[:])
            nc.sync.dma_start(pet, pet_d[:])
            nc.sync.dma_start(ind6, ind6_d[:])
            zpad = cpool.tile([H, 2], DT)
            nc.sync.dma_start(zpad, zpad_d[:])
            if use_b2p:
                b2rep = cpool.tile([J, MID], F32)
                nc.sync.dma_start(b2rep, b2rep_d[:])

            # pre-zeroed padded-hT ping-pong tiles (pads written once)
            htps = []
            zb = zpad.unsqueeze(1).broadcast_to([H, GRP, 2])
            for k in range(2):
                t = dpool.tile([H, GRP, J], DT, tag=f"htp{k}", bufs=1)
                nc.vector.tensor_copy(t[:, :, 0:2], zb)
                nc.vector.tensor_copy(t[:, :, J - 2 : J], zb)
                htps.append(t)

            for gp in range(NGRP // 2):
                t0 = gp * 2 * TT
                xt2 = dpool.tile([IN + 1, 2, TT], DT, tag="xt")
                nc.sync.dma_start(
                    xt2.rearrange("p a b -> p (a b)"), xT[:, t0 : t0 + 2 * TT]
                )
                w5st = dpool.tile([6, 2, GRP, S], F32, tag="w5s")
                m2st = dpool.tile([GRP * OUT, 2, S], F32, tag="m2s")

                for k in range(2):
                    g = gp * 2 + k
                    xt = xt2[:, k, :]
                    htp = htps[g % 2]

                    # mm1: hT = relu(W1aug^T @ xT) (+pe below), feature-major
                    ph = ppool.tile([H, GRP, S], F32, tag="ph")
                    nc.tensor.matmul(ph, w1aug, xt, start=True, stop=True)
                    nc.scalar.activation(htp[:, :, 2 : 2 + S], ph, AF.Relu)
                    nc.vector.tensor_add(
                        htp[:, :, 2 : 2 + S],
                        htp[:, :, 2 : 2 + S],
                        pet.unsqueeze(1).broadcast_to([H, GRP, S]),
                    )

                    # mm2: gT = A^T @ hT
                    pg = ppool.tile([H, GRP, S], F32, tag="pg")
                    nc.tensor.matmul(
                        pg, amat, htp[:, :, 2 : 2 + S], start=True, stop=True
                    )
                    gt = dpool.tile([H, GRP, S], DT, tag="gt")
                    nc.vector.tensor_copy(gt, pg)

                    # mm3 (per seq): scoresT[j, s] = hpad[j] . g[s]
                    psc = ppool.tile([J, GRP, S], F32, tag="psc", bufs=2)
                    for i in range(GRP):
                        nc.tensor.matmul(
                            psc[:, i, :], htp[:, i, :], gt[:, i, :],
                            start=True, stop=True,
                        )
                    # exp (scores bounded ~11, no max-sub needed), then band
                    # mask: keep iff s <= j <= s+4 (two gpsimd affine selects)
                    expa = dpool.tile([J, GRP, S], DT, tag="expa")
                    expb = dpool.tile([J, GRP, S], DT, tag="expb")
                    nc.scalar.activation(expa, psc, AF.Exp)
                    nc.gpsimd.affine_select(
                        expb, expa, pattern=[[0, GRP], [1, S]],
                        compare_op=ALU.is_ge, fill=0.0,
                        base=4, channel_multiplier=-1,
                    )
                    nc.gpsimd.affine_select(
                        expa, expb, pattern=[[0, GRP], [-1, S]],
                        compare_op=ALU.is_ge, fill=0.0,
                        base=0, channel_multiplier=1,
                    )

                    # mm4: W5 = Ind6^T @ expT (5 permuted band rows + denom)
                    pw5 = ppool.tile([6, GRP, S], F32, tag="pw5")
                    nc.tensor.matmul(
                        pw5, ind6, expa.rearrange("p a b -> p (a b)"),
                        start=True, stop=True,
                    )
                    nc.vector.tensor_copy(w5st[:, k], pw5)

                    # mm5 (per seq): hw2[j, mid] = hpad[j] @ W2p
                    phw2 = ppool.tile([J, GRP * MID], F32, tag="phw2")
                    for i in range(GRP):
                        nc.tensor.matmul(
                            phw2[:, i * MID : (i + 1) * MID],
                            htp[:, i, :].bitcast(F32), w2p,
                            start=True, stop=True,
                        )
                    hw2 = dpool.tile([J, GRP * MID], F32, tag="hw2")
                    nc.scalar.copy(hw2, phw2)

                    # mm6 (per seq, col-packed): mlp1T[mid, s] = hw2^T @ expT
                    pm1 = ppool.tile([GRP * MID, S], F32, tag="pm1")
                    for i in range(GRP):
                        nc.tensor.matmul(
                            pm1[i * MID : (i + 1) * MID, :],
                            hw2[:, i * MID : (i + 1) * MID],
                            expa[:, i, :].bitcast(F32),
                            start=True, stop=not use_b2p,
                            tile_position=(0, i * MID),
                        )
                        if use_b2p:
                            # += b2p[mid]*denom[s]: host /denom yields +b2p
                            nc.tensor.matmul(
                                pm1[i * MID : (i + 1) * MID, :],
                                b2rep, expa[:, i, :].bitcast(F32),
                                start=False, stop=True,
                                tile_position=(0, i * MID),
                            )
                    r1 = dpool.tile([GRP * MID, S], F32, tag="r1")
                    nc.scalar.activation(r1, pm1, AF.Relu)

                    # mm7: m2 = W3blk^T @ r1 -> [GRP*OUT, S]
                    pm2 = ppool.tile([GRP * OUT, S], F32, tag="pm2")
                    nc.tensor.matmul(pm2, w3blk, r1, start=True, stop=True)
                    nc.scalar.copy(m2st[:, k], pm2)

                nc.sync.dma_start(
                    w5_d[2 * gp : 2 * gp + 2].rearrange("g r s -> r g s"),
                    w5st.rearrange("p g a b -> p g (a b)"),
                )
                nc.sync.dma_start(
                    m2_d[2 * gp : 2 * gp + 2].rearrange("g p s -> p g s"), m2st
                )

    nc.compile()
    return nc


_CACHE = {}


USE_F32R = True


def _get_nc(use_b2p: bool):
    key = (use_b2p, USE_F32R)
    if key not in _CACHE:
        _CACHE[key] = build_nc(use_b2p, USE_F32R)
    return _CACHE[key]


def _host_prep(inputs):
    x = np.asarray(inputs["x"], np.float32)
    W1 = np.asarray(inputs["W1"], np.float32)
    b1 = np.asarray(inputs["b1"], np.float32).reshape(H)
    Wq = np.asarray(inputs["Wq"], np.float32)
    Wk = np.asarray(inputs["Wk"], np.float32)
    Wv = np.asarray(inputs["Wv"], np.float32)
    bv = np.asarray(inputs["bv"], np.float32).reshape(H)
    W2 = np.asarray(inputs["W2"], np.float32)
    b2 = np.asarray(inputs["b2"], np.float32).reshape(MID)
    W3 = np.asarray(inputs["W3"], np.float32)
    b3 = np.asarray(inputs["b3"], np.float32).reshape(OUT)
    pe = np.asarray(inputs["pe"], np.float32)[0]

    A = (Wq @ Wk.T) / SQRT_H
    W2p = Wv @ W2
    b2p = bv @ W2 + b2
    W1aug = np.ascontiguousarray(np.concatenate([W1, b1[None, :]], 0))
    W3blk = np.zeros((H, GRP * OUT), np.float32)
    for i in range(GRP):
        W3blk[MID * i : MID * i + MID, OUT * i : OUT * i + OUT] = W3
    ind6 = np.zeros((J, 6), np.float32)
    ind6[np.arange(J), np.arange(J) % 5] = 1.0
    ind6[:, 5] = 1.0
    peT = np.ascontiguousarray(pe[:S].T)
    b2rep = np.ascontiguousarray(np.broadcast_to(b2p[None, :], (J, MID)))

    consts = {
        "zpad": np.zeros((H, 2), np.float32),
        "w1aug": W1aug,
        "amat": np.ascontiguousarray(A),
        "w2p": np.ascontiguousarray(W2p),
        "w3blk": W3blk,
        "pet": peT,
        "ind6": ind6,
        "b2rep": b2rep,
    }
    use_b2p = bool(np.any(b2p != 0.0))

    in_maps = []
    for c in range(NCORES):
        xs = x[c * BS : (c + 1) * BS].reshape(BS * S, IN).T
        xT_aug = np.ascontiguousarray(
            np.concatenate([xs, np.ones((1, BS * S), np.float32)], 0)
        )
        m = {"xT": xT_aug}
        m.update(consts)
        in_maps.append(m)
    return in_maps, use_b2p, b3


# index table undoing the mod-5 residue permutation: w[s, m] = W5[(s+4-m)%5, s]
_RIDX = ((np.arange(S)[:, None] + 4 - np.arange(5)[None, :]) % 5).astype(np.int64)


def _host_post(results, b3):
    # per core: w5 [NGRP, 6, TT], m2 [NGRP, GRP*OUT, S]
    w5 = np.stack([np.asarray(r["w5"]) for r in results])  # [NC, NGRP, 6, TT]
    m2 = np.stack([np.asarray(r["m2"]) for r in results])  # [NC, NGRP, 8, S]

    w5 = w5.reshape(NCORES, NGRP, 6, GRP, S).transpose(0, 1, 3, 2, 4)
    w5 = w5.reshape(B, 6, S)                     # [b, r, s]
    denom = w5[:, 5, :]                          # [b, s]
    # band: w[b, s, m] = w5[b, (s+4-m)%5, s] / denom
    sidx = np.arange(S)[:, None]
    wband = w5[:, _RIDX, sidx] / denom[:, :, None]   # [B, S, 5]

    m2 = m2.reshape(NCORES, NGRP, GRP, OUT, S).transpose(0, 1, 2, 4, 3)
    m2 = m2.reshape(B, S, OUT)
    out = m2 / denom[:, :, None] + b3[None, None, :]

    return out.astype(np.float32), wband.astype(np.float32)


def kernel(**inputs):
    in_maps, use_b2p, b3 = _host_prep(inputs)
    nc = _get_nc(use_b2p)
    res = run_bass_kernel_spmd(nc, in_maps, list(range(NCORES)))
    out, wband = _host_post(res.results, b3)
    return out, wband[:, :, None, :]


# revision 11
# speedup vs baseline: 1.0474x; 1.0045x over previous
"""Trainium2 Bass kernel for nn_ExRestSelfAtten (windowed local attention).

Self-contained: hardcodes shapes, shards batch across 8 NeuronCores (pure data
parallel), runs a Tile/Bass kernel per core, reassembles full outputs on host.

Math (validated vs reference in fp64/fp32 numpy, err ~3e-6):
  h  = relu(x@W1 + b1) + pe                      (feature-major hT on device)
  scores(s,s') = (h[s]@A) . h[s'],  A = Wq Wk^T / sqrt(H)
  banded softmax over s' in [s-2, s+2] with zero-padding semantics
  context @ Wv folded:  ctx_h = sum_m w_m h[s+2-m];  W2' = Wv@W2
  out = relu(ctx@W2' + b2')@W3 + b3
Key trick: with masked exp in j-major layout expT[j,s] (zeros off-band), the
5-band extraction is a matmul with Ind[j,r] = [j==r mod 5] (5 consecutive
in-band j's have distinct residues mod 5); a ones column gives the softmax
denominator. The per-row cyclic permutation of the 5 residues and the final
division by the denominator are undone on the host at gather time.
"""

import numpy as np

import concourse.bacc as bacc
import concourse.bass as bass
import concourse.mybir as mybir
from concourse import tile
from concourse.bass_utils import run_bass_kernel_spmd

F32 = mybir.dt.float32
F32R = mybir.dt.float32r

B, S, IN, H, MID, OUT = 1024, 100, 100, 128, 32, 2
ATTEN = 2
J = S + 2 * ATTEN          # 104 padded neighbor positions
NCORES = 8
BS = B // NCORES           # 128 sequences per core
GRP = 4                    # sequences per inner group (one PSUM bank of scores)
NGRP = BS // GRP           # 32 groups per core
TT = GRP * S               # 400 tokens per group
SQRT_H = float(np.sqrt(float(H)))

AF = mybir.ActivationFunctionType
ALU = mybir.AluOpType


def build_nc(use_b2p: bool, use_f32r: bool = True):
    DT = F32R if use_f32r else F32
    nc = bacc.Bacc("TRN2", num_devices=NCORES)

    xT = nc.dram_tensor("xT", [IN + 1, BS * S], DT, kind="ExternalInput")
    w1aug_d = nc.dram_tensor("w1aug", [IN + 1, H], DT, kind="ExternalInput")
    a_d = nc.dram_tensor("amat", [H, H], DT, kind="ExternalInput")
    w2p_d = nc.dram_tensor("w2p", [H, MID], F32, kind="ExternalInput")
    w3blk_d = nc.dram_tensor("w3blk", [H, GRP * OUT], F32, kind="ExternalInput")
    pet_d = nc.dram_tensor("pet", [H, S], F32, kind="ExternalInput")
    ind6_d = nc.dram_tensor("ind6", [J, 6], DT, kind="ExternalInput")
    zpad_d = nc.dram_tensor("zpad", [H, 2], DT, kind="ExternalInput")
    b2rep_d = nc.dram_tensor("b2rep", [J, MID], F32, kind="ExternalInput")

    w5_d = nc.dram_tensor("w5", [NGRP, 6, TT], F32, kind="ExternalOutput")
    m2_d = nc.dram_tensor("m2", [NGRP, GRP * OUT, S], F32, kind="ExternalOutput")

    with tile.TileContext(nc) as tc:
        with (
            tc.tile_pool(name="consts", bufs=1) as cpool,
            tc.tile_pool(name="data", bufs=2) as dpool,
            tc.tile_pool(name="psum", bufs=1, space="PSUM") as ppool,
        ):
            w1aug = cpool.tile([IN + 1, H], DT)
            amat = cpool.tile([H, H], DT)
            w2p = cpool.tile([H, MID], F32)
            w3blk = cpool.tile([H, GRP * OUT], F32)
            pet = cpool.tile([H, S], F32)
            ind6 = cpool.tile([J, 6], DT)
            nc.sync.dma_start(w1aug, w1aug_d[:])
            nc.sync.dma_start(amat, a_d[:])
            nc.sync.dma_start(w2p, w2p_d[:])
            nc.sync.dma_start(w3blk, w3blk_d[:])
            nc.sync.dma_start(pet, pet_d[:])
            nc.sync.dma_start(ind6, ind6_d[:])
            zpad = cpool.tile([H, 2], DT)
            nc.sync.dma_start(zpad, zpad_d[:])
            if use_b2p:
                b2rep = cpool.tile([J, MID], F32)
                nc.sync.dma_start(b2rep, b2rep_d[:])

            # pre-zeroed padded-hT ping-pong tiles (pads written once)
            htps = []
            zb = zpad.unsqueeze(1).broadcast_to([H, GRP, 2])
            for k in range(2):
                t = dpool.tile([H, GRP, J], DT, tag=f"htp{k}", bufs=1)
                nc.vector.tensor_copy(t[:, :, 0:2], zb)
                nc.vector.tensor_copy(t[:, :, J - 2 : J], zb)
                htps.append(t)

            # Software pipeline: stage A (mm1/mm2/scores/exp/mask/hw2) of
            # group g+1 is issued before stage B (band/mlp) of group g, so
            # the PE always has independent work while the ACT exp + gpsimd
            # mask chain of the current group completes (keeps HAM warm).

            def stage_a(xt, htp):
                # mm1: hT = relu(W1aug^T @ xT) (+pe below), feature-major
                ph = ppool.tile([H, GRP, S], F32, tag="pa", bufs=2)
                nc.tensor.matmul(ph, w1aug, xt, start=True, stop=True)
                nc.scalar.activation(htp[:, :, 2 : 2 + S], ph, AF.Relu)
                nc.vector.tensor_add(
                    htp[:, :, 2 : 2 + S],
                    htp[:, :, 2 : 2 + S],
                    pet.unsqueeze(1).broadcast_to([H, GRP, S]),
                )
                # mm2: gT = A^T @ hT
                pg = ppool.tile([H, GRP, S], F32, tag="pa", bufs=2)
                nc.tensor.matmul(
                    pg, amat, htp[:, :, 2 : 2 + S], start=True, stop=True
                )
                gt = dpool.tile([H, GRP, S], DT, tag="gt")
                nc.vector.tensor_copy(gt, pg)

                # mm3 (per seq): scoresT[j, s] = hpad[j] . g[s]
                psc = ppool.tile([J, GRP, S], F32, tag="psc", bufs=2)
                for i in range(GRP):
                    nc.tensor.matmul(
                        psc[:, i, :], htp[:, i, :], gt[:, i, :],
                        start=True, stop=True,
                    )
                # mm5 (per seq): hw2[j, mid] = hpad[j] @ W2p
                phw2 = ppool.tile([J, GRP * MID], F32, tag="phw2", bufs=2)
                for i in range(GRP):
                    nc.tensor.matmul(
                        phw2[:, i * MID : (i + 1) * MID],
                        htp[:, i, :].bitcast(F32), w2p,
                        start=True, stop=True,
                    )
                hw2 = dpool.tile([J, GRP * MID], F32, tag="hw2")
                nc.scalar.copy(hw2, phw2)

                # exp (scores bounded ~11, no max-sub needed), then band
                # mask: keep iff s <= j <= s+4 (two gpsimd affine selects)
                expa = dpool.tile([J, GRP, S], DT, tag="expa")
                expb = dpool.tile([J, GRP, S], DT, tag="expb")
                nc.scalar.activation(expa, psc, AF.Exp)
                nc.gpsimd.affine_select(
                    expb, expa, pattern=[[0, GRP], [1, S]],
                    compare_op=ALU.is_ge, fill=0.0,
                    base=4, channel_multiplier=-1,
                )
                nc.gpsimd.affine_select(
                    expa, expb, pattern=[[0, GRP], [-1, S]],
                    compare_op=ALU.is_ge, fill=0.0,
                    base=0, channel_multiplier=1,
                )
                return expa, hw2

            def stage_b(st):
                expa, hw2, w5st, m2st, k, gp = st
                # mm4: W5 = Ind6^T @ expT (5 permuted band rows + denom)
                pw5 = ppool.tile([6, GRP, S], F32, tag="pout", bufs=1)
                nc.tensor.matmul(
                    pw5, ind6, expa.rearrange("p a b -> p (a b)"),
                    start=True, stop=True,
                )
                nc.vector.tensor_copy(w5st[:, k], pw5)

                # mm6 (per seq, col-packed): mlp1T[mid, s] = hw2^T @ expT
                pm1 = ppool.tile([GRP * MID, S], F32, tag="pm1", bufs=1)
                for i in range(GRP):
                    nc.tensor.matmul(
                        pm1[i * MID : (i + 1) * MID, :],
                        hw2[:, i * MID : (i + 1) * MID],
                        expa[:, i, :].bitcast(F32),
                        start=True, stop=not use_b2p,
                        tile_position=(0, i * MID),
                    )
                    if use_b2p:
                        # += b2p[mid]*denom[s]: host /denom yields +b2p
                        nc.tensor.matmul(
                            pm1[i * MID : (i + 1) * MID, :],
                            b2rep, expa[:, i, :].bitcast(F32),
                            start=False, stop=True,
                            tile_position=(0, i * MID),
                        )
                r1 = dpool.tile([GRP * MID, S], F32, tag="r1")
                nc.scalar.activation(r1, pm1, AF.Relu)

                # mm7: m2 = W3blk^T @ r1 -> [GRP*OUT, S]
                pm2 = ppool.tile([GRP * OUT, S], F32, tag="pout", bufs=1)
                nc.tensor.matmul(pm2, w3blk, r1, start=True, stop=True)
                nc.scalar.copy(m2st[:, k], pm2)

                if k == 1:
                    nc.sync.dma_start(
                        w5_d[2 * gp : 2 * gp + 2].rearrange("g r s -> r g s"),
                        w5st.rearrange("p g a b -> p g (a b)"),
                    )
                    nc.sync.dma_start(
                        m2_d[2 * gp : 2 * gp + 2].rearrange("g p s -> p g s"),
                        m2st,
                    )

            pend = None
            for gp in range(NGRP // 2):
                t0 = gp * 2 * TT
                xt2 = dpool.tile([IN + 1, 2, TT], DT, tag="xt")
                nc.sync.dma_start(
                    xt2.rearrange("p a b -> p (a b)"), xT[:, t0 : t0 + 2 * TT]
                )
                w5st = dpool.tile([6, 2, GRP, S], F32, tag="w5s")
                m2st = dpool.tile([GRP * OUT, 2, S], F32, tag="m2s")
                for k in range(2):
                    g = gp * 2 + k
                    expa, hw2 = stage_a(xt2[:, k, :], htps[g % 2])
                    if pend is not None:
                        stage_b(pend)
                    pend = (expa, hw2, w5st, m2st, k, gp)
            stage_b(pend)

    nc.compile()
    return nc


_CACHE = {}


USE_F32R = True


def _get_nc(use_b2p: bool):
    key = (use_b2p, USE_F32R)
    if key not in _CACHE:
        _CACHE[key] = build_nc(use_b2p, USE_F32R)
    return _CACHE[key]


def _host_prep(inputs):
    x = np.asarray(inputs["x"], np.float32)
    W1 = np.asarray(inputs["W1"], np.float32)
    b1 = np.asarray(inputs["b1"], np.float32).reshape(H)
    Wq = np.asarray(inputs["Wq"], np.float32)
    Wk = np.asarray(inputs["Wk"], np.float32)
    Wv = np.asarray(inputs["Wv"], np.float32)
    bv = np.asarray(inputs["bv"], np.float32).reshape(H)
    W2 = np.asarray(inputs["W2"], np.float32)
    b2 = np.asarray(inputs["b2"], np.float32).reshape(MID)
    W3 = np.asarray(inputs["W3"], np.float32)
    b3 = np.asarray(inputs["b3"], np.float32).reshape(OUT)
    pe = np.asarray(inputs["pe"], np.float32)[0]

    A = (Wq @ Wk.T) / SQRT_H
    W2p = Wv @ W2
    b2p = bv @ W2 + b2
    W1aug = np.ascontiguousarray(np.concatenate([W1, b1[None, :]], 0))
    W3blk = np.zeros((H, GRP * OUT), np.float32)
    for i in range(GRP):
        W3blk[MID * i : MID * i + MID, OUT * i : OUT * i + OUT] = W3
    ind6 = np.zeros((J, 6), np.float32)
    ind6[np.arange(J), np.arange(J) % 5] = 1.0
    ind6[:, 5] = 1.0
    peT = np.ascontiguousarray(pe[:S].T)
    b2rep = np.ascontiguousarray(np.broadcast_to(b2p[None, :], (J, MID)))

    consts = {
        "zpad": np.zeros((H, 2), np.float32),
        "w1aug": W1aug,
        "amat": np.ascontiguousarray(A),
        "w2p": np.ascontiguousarray(W2p),
        "w3blk": W3blk,
        "pet": peT,
        "ind6": ind6,
        "b2rep": b2rep,
    }
    use_b2p = bool(np.any(b2p != 0.0))

    in_maps = []
    for c in range(NCORES):
        xs = x[c * BS : (c + 1) * BS].reshape(BS * S, IN).T
        xT_aug = np.ascontiguousarray(
            np.concatenate([xs, np.ones((1, BS * S), np.float32)], 0)
        )
        m = {"xT": xT_aug}
        m.update(consts)
        in_maps.append(m)
    return in_maps, use_b2p, b3


# index table undoing the mod-5 residue permutation: w[s, m] = W5[(s+4-m)%5, s]
_RIDX = ((np.arange(S)[:, None] + 4 - np.arange(5)[None, :]) % 5).astype(np.int64)


def _host_post(results, b3):
    # per core: w5 [NGRP, 6, TT], m2 [NGRP, GRP*OUT, S]
    w5 = np.stack([np.asarray(r["w5"]) for r in results])  # [NC, NGRP, 6, TT]
    m2 = np.stack([np.asarray(r["m2"]) for r in results])  # [NC, NGRP, 8, S]

    w5 = w5.reshape(NCORES, NGRP, 6, GRP, S).transpose(0, 1, 3, 2, 4)
    w5 = w5.reshape(B, 6, S)                     # [b, r, s]
    denom = w5[:, 5, :]                          # [b, s]
    # band: w[b, s, m] = w5[b, (s+4-m)%5, s] / denom
    sidx = np.arange(S)[:, None]
    wband = w5[:, _RIDX, sidx] / denom[:, :, None]   # [B, S, 5]

    m2 = m2.reshape(NCORES, NGRP, GRP, OUT, S).transpose(0, 1, 2, 4, 3)
    m2 = m2.reshape(B, S, OUT)
    out = m2 / denom[:, :, None] + b3[None, None, :]

    return out.astype(np.float32), wband.astype(np.float32)


def kernel(**inputs):
    in_maps, use_b2p, b3 = _host_prep(inputs)
    nc = _get_nc(use_b2p)
    res = run_bass_kernel_spmd(nc, in_maps, list(range(NCORES)))
    out, wband = _host_post(res.results, b3)
    return out, wband[:, :, None, :]
